# revision 21
# baseline (speedup 1.0000x reference)
"""Trainium2 Bass kernel for AllLayerXSA (cross-layer sparse attention).

Math (per token t):
  qk[t,:]    = x15[t,:] @ M,  M = (W_q^T @ W_k) / (sqrt(D) * |temp|)   (host-precomputed)
  scores[t,l]= qk[t,:] . x_l[t,:]
  u = exp(scores); v = u*s; w = v*s / (sum(v) + 1e-6*sum(u))
  out[t,:]   = sum_l w[t,l] * x_l[t,:]       (PE: diag(w_l) @ X_l, PSUM-accumulated)
  y = LayerNorm(out) * ln_w + ln_b

Sharding: tokens (B*T = 4096) split evenly across 8 cores; weights replicated.
Software-pipelined: block n+1's loads/transpose/qk are issued before block n's
weighted-sum so the in-order PE queue never stalls DVE.
"""

import sys

if "/opt/trn_rl_repo" not in sys.path:
    sys.path.insert(0, "/opt/trn_rl_repo")

import numpy as np
import ml_dtypes

import concourse.bacc as bacc
import concourse.mybir as mybir
from concourse import tile
from concourse.bass_utils import run_bass_kernel_spmd

L, B, T, D = 16, 2, 2048, 1024
N_CORES = 8
NT = (B * T) // N_CORES  # tokens per core
NBLK = NT // 128  # 128-token blocks per core

F32 = mybir.dt.float32
F32R = mybir.dt.float32r
I32 = mybir.dt.int32
AF = mybir.ActivationFunctionType
ALU = mybir.AluOpType
AX = mybir.AxisListType


def build_nc(qk_f32r=True, ws_f32r=True):
    XDT = F32R if ws_f32r else F32  # x tiles feed the f32r weighted-sum matmuls
    MDT = F32R if qk_f32r else F32
    nc = bacc.Bacc("TRN2", target_bir_lowering=False, debug=False)
    x_d = nc.dram_tensor("x", [L, NT, D], XDT, kind="ExternalInput").ap()
    m_d = nc.dram_tensor("m", [D, D], MDT, kind="ExternalInput").ap()
    sc_d = nc.dram_tensor("scales_b", [128, L], F32, kind="ExternalInput").ap()
    id_d = nc.dram_tensor("ident", [128, 128], F32, kind="ExternalInput").ap()
    lnw_d = nc.dram_tensor("lnw_b", [128, D], F32, kind="ExternalInput").ap()
    lnb_d = nc.dram_tensor("lnb_b", [128, D], F32, kind="ExternalInput").ap()
    y_d = nc.dram_tensor("y", [NT, D], F32, kind="ExternalOutput").ap()

    with tile.TileContext(nc) as tc:
        with (
            tc.tile_pool(name="const", bufs=1) as constp,
            tc.tile_pool(name="x", bufs=22) as xp,
            tc.tile_pool(name="x15T", bufs=2) as tp15,
            tc.tile_pool(name="qks", bufs=2) as qksp,
            tc.tile_pool(name="scr", bufs=1) as scrp,
            tc.tile_pool(name="sm", bufs=2) as smp,
            tc.tile_pool(name="diag", bufs=2) as diagp,
            tc.tile_pool(name="z", bufs=2) as zp,
            tc.tile_pool(name="y", bufs=2) as yp,
            tc.tile_pool(name="tr_ps", bufs=2, space="PSUM") as tr_ps,
            tc.tile_pool(name="qk_ps", bufs=1, space="PSUM") as qk_ps,
            tc.tile_pool(name="o_ps", bufs=2, space="PSUM") as o_ps,
        ):
            # ---- constants ----
            m_sb = constp.tile([128, 8, D], MDT)
            nc.sync.dma_start(m_sb[:], m_d.rearrange("(c p) e -> p c e", p=128))
            sc_sb = constp.tile([128, L], F32)
            nc.sync.dma_start(sc_sb[:], sc_d[:])
            id_sb = constp.tile([128, 128], F32)
            nc.sync.dma_start(id_sb[:], id_d[:])
            lnw_sb = constp.tile([128, D], F32)
            nc.sync.dma_start(lnw_sb[:], lnw_d[:])
            lnb_sb = constp.tile([128, D], F32)
            nc.sync.dma_start(lnb_sb[:], lnb_d[:])
            scratch = scrp.tile([128, D], F32, tag="scratch")
            scratch2 = scrp.tile([128, D], F32, tag="scratch2")
            magic = constp.tile([128, 1], I32, tag="magic")
            nc.vector.memset(magic[:], 0x5F3759DF)

            def front(b):
                """Block-front: x loads, x15 transpose, qk matmul (PE-early)."""
                r0 = b * 128
                x15 = xp.tile([128, D], XDT, tag="x")
                nc.sync.dma_start(x15[:], x_d[L - 1, r0 : r0 + 128, :])
                xt = []
                for l in range(L - 1):
                    t_ = xp.tile([128, D], XDT, tag="x")
                    nc.sync.dma_start(t_[:], x_d[l, r0 : r0 + 128, :])
                    xt.append(t_)
                xt.append(x15)

                x15T = tp15.tile([128, 8, 128], MDT)
                for c in range(8):
                    trp = tr_ps.tile([128, 128], F32, tag="tr")
                    nc.tensor.transpose(
                        trp[:], x15[:, c * 128 : (c + 1) * 128].bitcast(F32), id_sb[:]
                    )
                    nc.scalar.activation(x15T[:, c, :], trp[:], AF.Identity)

                qkp = qk_ps.tile([128, D], F32)
                for c in range(8):
                    for h in range(2):
                        nc.tensor.matmul(
                            qkp[:, h * 512 : (h + 1) * 512],
                            x15T[:, c, :],
                            m_sb[:, c, h * 512 : (h + 1) * 512],
                            start=(c == 0),
                            stop=(c == 7),
                        )
                qk_sb = qksp.tile([128, D], F32)
                nc.scalar.activation(qk_sb[:], qkp[:], AF.Identity)
                return xt, qk_sb

            def back(b, xt, qk_sb):
                """Block-back: scores, softmax, weighted sum, LayerNorm, store."""
                r0 = b * 128
                # scores[t,l] = sum_d x_l[t,d]*qk[t,d]  (DVE fused mul+reduce)
                scores = smp.tile([128, L], F32, tag="scores")
                for l in range(L):
                    nc.vector.scalar_tensor_tensor(
                        out=scratch[:],
                        in0=xt[l][:].bitcast(F32),
                        scalar=1.0,
                        in1=qk_sb[:],
                        op0=ALU.mult,
                        op1=ALU.mult,
                        accum_out=scores[:, l : l + 1],
                    )

                # softmax over l + double scaling + renorm
                u = smp.tile([128, L], F32, tag="u")
                Ucol = smp.tile([128, 1], F32, tag="Ucol")
                nc.scalar.activation(u[:], scores[:], AF.Exp, accum_out=Ucol[:])
                v = smp.tile([128, L], F32, tag="v")
                nc.vector.tensor_mul(v[:], u[:], sc_sb[:])
                Vcol = smp.tile([128, 1], F32, tag="Vcol")
                nc.vector.tensor_reduce(Vcol[:], v[:], axis=AX.X, op=ALU.add)
                den = smp.tile([128, 1], F32, tag="den")
                nc.vector.tensor_scalar(
                    out=den[:], in0=Ucol[:], scalar1=1e-6, scalar2=Vcol[:],
                    op0=ALU.mult, op1=ALU.add,
                )
                rden = smp.tile([128, 1], F32, tag="rden")
                nc.vector.reciprocal(rden[:], den[:])
                w_t = smp.tile([128, L], F32, tag="w")
                nc.vector.scalar_tensor_tensor(
                    out=w_t[:], in0=v[:], scalar=rden[:], in1=sc_sb[:],
                    op0=ALU.mult, op1=ALU.mult,
                )

                # all 16 diag(w_l) in one broadcast tensor_tensor
                dg_all = diagp.tile([128, L, 128], XDT, tag="diag")
                nc.vector.tensor_tensor(
                    out=dg_all[:],
                    in0=id_sb[:].unsqueeze(1).broadcast_to((128, L, 128)),
                    in1=w_t[:].unsqueeze(2).broadcast_to((128, L, 128)),
                    op=ALU.mult,
                )
                op_ = o_ps.tile([128, D], F32)
                for l in range(L):
                    for h in range(2):
                        nc.tensor.matmul(
                            op_[:, h * 512 : (h + 1) * 512],
                            dg_all[:, l, :],
                            xt[l][:, h * 512 : (h + 1) * 512],
                            start=(l == 0),
                            stop=(l == L - 1),
                        )

                # LayerNorm stats: sum on ACT (identity-accum), sumsq on ACT (square-accum)
                s1 = smp.tile([128, 1], F32, tag="s1")
                nc.scalar.activation(scratch2[:], op_[:], AF.Identity, accum_out=s1[:])
                s2 = smp.tile([128, 1], F32, tag="s2")
                nc.scalar.activation(scratch2[:], op_[:], AF.Square, accum_out=s2[:])
                m1 = smp.tile([128, 1], F32, tag="m1")
                nc.vector.tensor_scalar(
                    out=m1[:], in0=s1[:], scalar1=1.0 / D, scalar2=None, op0=ALU.mult
                )
                q2 = smp.tile([128, 1], F32, tag="q2")
                nc.vector.tensor_scalar(
                    out=q2[:], in0=s2[:], scalar1=1.0 / D, scalar2=None, op0=ALU.mult
                )
                m1sq = smp.tile([128, 1], F32, tag="m1sq")
                nc.vector.tensor_scalar(
                    out=m1sq[:], in0=m1[:], scalar1=m1[:], scalar2=None, op0=ALU.mult
                )
                var_ = smp.tile([128, 1], F32, tag="var")  # var + eps
                nc.vector.scalar_tensor_tensor(
                    out=var_[:], in0=q2[:], scalar=1e-5, in1=m1sq[:],
                    op0=ALU.add, op1=ALU.subtract,
                )
                # rstd = 1/sqrt(var+eps): fast-inverse-sqrt + 3 Newton steps (DVE only)
                rstd = smp.tile([128, 1], F32, tag="rstd")
                ti = smp.tile([128, 1], I32, tag="nt_ti")
                nc.vector.tensor_scalar(
                    out=ti[:], in0=var_[:].bitcast(I32), scalar1=1,
                    scalar2=None, op0=ALU.arith_shift_right,
                )
                nc.vector.scalar_tensor_tensor(
                    out=rstd[:].bitcast(I32), in0=magic[:], scalar=0.0,
                    in1=ti[:], op0=ALU.bypass, op1=ALU.subtract,
                )
                for _ in range(2):
                    a_ = smp.tile([128, 1], F32, tag="nt_a")
                    nc.vector.tensor_scalar(
                        out=a_[:], in0=rstd[:], scalar1=rstd[:], scalar2=None,
                        op0=ALU.mult,
                    )
                    b_ = smp.tile([128, 1], F32, tag="nt_b")
                    nc.vector.scalar_tensor_tensor(
                        out=b_[:], in0=var_[:], scalar=-0.5, in1=a_[:],
                        op0=ALU.mult, op1=ALU.mult,
                    )
                    c_ = smp.tile([128, 1], F32, tag="nt_c")
                    nc.vector.tensor_scalar(
                        out=c_[:], in0=b_[:], scalar1=1.5, scalar2=None, op0=ALU.add
                    )
                    rstd_new = smp.tile([128, 1], F32, tag="nt_y")
                    nc.vector.tensor_scalar(
                        out=rstd_new[:], in0=rstd[:], scalar1=c_[:], scalar2=None,
                        op0=ALU.mult,
                    )
                    rstd = rstd_new
                nmr = smp.tile([128, 1], F32, tag="nmr")  # -mean*rstd
                nc.vector.tensor_scalar(
                    out=nmr[:], in0=m1[:], scalar1=rstd[:], scalar2=-1.0,
                    op0=ALU.mult, op1=ALU.mult,
                )
                z = zp.tile([128, D], F32)
                nc.scalar.activation(z[:], op_[:], AF.Identity, scale=rstd[:], bias=nmr[:])
                y1 = yp.tile([128, D], F32, tag="y1")
                nc.gpsimd.tensor_mul(y1[:], z[:], lnw_sb[:])
                nc.gpsimd.tensor_add(y1[:], y1[:], lnb_sb[:])
                nc.sync.dma_start(y_d[r0 : r0 + 128, :], y1[:])

            # software pipeline: front(n+1) issued before back(n)
            state = front(0)
            for b in range(NBLK):
                nxt = front(b + 1) if b + 1 < NBLK else None
                back(b, *state)
                state = nxt

    nc.compile()
    return nc


def build_nc_f16(n_stt=5):
    """fp16-input variant: x/M cast to fp16 on host. Scores use a DVE/ACT
    split: n_stt layers as fused scalar_tensor_tensor on DVE, the rest as
    fp16 2x tensor_tensor multiply on DVE + Identity-accumulate reduce on ACT.
    """
    BF = mybir.dt.float16
    nc = bacc.Bacc("TRN2", target_bir_lowering=False, debug=False)
    x_d = nc.dram_tensor("x", [L, NT, D], BF, kind="ExternalInput").ap()
    m_d = nc.dram_tensor("m", [D, D], BF, kind="ExternalInput").ap()
    sc_d = nc.dram_tensor("scales_b", [128, L], F32, kind="ExternalInput").ap()
    id_d = nc.dram_tensor("ident", [128, 128], F32, kind="ExternalInput").ap()
    idb_d = nc.dram_tensor("ident_bf", [128, 128], BF, kind="ExternalInput").ap()
    lnw_d = nc.dram_tensor("lnw_b", [128, D], F32, kind="ExternalInput").ap()
    lnb_d = nc.dram_tensor("lnb_b", [128, D], F32, kind="ExternalInput").ap()
    y_d = nc.dram_tensor("y", [NT, D], F32, kind="ExternalOutput").ap()

    with tile.TileContext(nc) as tc:
        with (
            tc.tile_pool(name="const", bufs=1) as constp,
            tc.tile_pool(name="x", bufs=52) as xp,
            tc.tile_pool(name="x15T", bufs=2) as tp15,
            tc.tile_pool(name="qks", bufs=2) as qksp,
            tc.tile_pool(name="scr", bufs=1) as scrp,
            tc.tile_pool(name="mulr", bufs=4) as mulp,
            tc.tile_pool(name="sm", bufs=2) as smp,
            tc.tile_pool(name="diag", bufs=2) as diagp,
            tc.tile_pool(name="z", bufs=2) as zp,
            tc.tile_pool(name="y", bufs=2) as yp,
            tc.tile_pool(name="tr_ps", bufs=2, space="PSUM") as tr_ps,
            tc.tile_pool(name="qk_ps", bufs=1, space="PSUM") as qk_ps,
            tc.tile_pool(name="o_ps", bufs=2, space="PSUM") as o_ps,
        ):
            # early consts (needed by block 0 front)
            sc_sb = constp.tile([128, L], F32)
            nc.sync.dma_start(sc_sb[:], sc_d[:])
            id_sb = constp.tile([128, 128], F32)
            nc.sync.dma_start(id_sb[:], id_d[:])
            idb_sb = constp.tile([128, 128], BF)
            nc.sync.dma_start(idb_sb[:], idb_d[:])
            m_sb = constp.tile([128, 8, D], BF)
            nc.sync.dma_start(m_sb[:], m_d.rearrange("(c p) e -> p c e", p=128))
            scratch = scrp.tile([128, D], F32, tag="scratch")
            act_dump = scrp.tile([128, D], BF, tag="act_dump")
            scratch2 = scrp.tile([128, D], F32, tag="scratch2")
            magic = constp.tile([128, 1], I32, tag="magic")
            nc.vector.memset(magic[:], 0x5F3759DF)

            def front(b):
                r0 = b * 128
                x15 = xp.tile([128, D], BF, tag="x")
                nc.sync.dma_start(x15[:], x_d[L - 1, r0 : r0 + 128, :])
                xt = []
                for l in range(L - 1):
                    t_ = xp.tile([128, D], BF, tag="x")
                    nc.sync.dma_start(t_[:], x_d[l, r0 : r0 + 128, :])
                    xt.append(t_)
                xt.append(x15)

                x15T = tp15.tile([128, 8, 128], BF)
                for g in range(2):
                    trp = tr_ps.tile([128, 4, 128], BF, tag="tr")
                    for j in range(4):
                        c = g * 4 + j
                        nc.tensor.transpose(
                            trp[:, j, :], x15[:, c * 128 : (c + 1) * 128], idb_sb[:]
                        )
                    nc.vector.tensor_copy(x15T[:, g * 4 : (g + 1) * 4, :], trp[:])

                qkp = qk_ps.tile([128, D], F32)
                for c in range(8):
                    for h in range(2):
                        nc.tensor.matmul(
                            qkp[:, h * 512 : (h + 1) * 512],
                            x15T[:, c, :],
                            m_sb[:, c, h * 512 : (h + 1) * 512],
                            start=(c == 0),
                            stop=(c == 7),
                        )
                qk_sb = qksp.tile([128, D], BF)
                nc.scalar.activation(qk_sb[:], qkp[:], AF.Identity)
                return xt, qk_sb

            def back(b, xt, qk_sb, last=False):
                r0 = b * 128
                scores = smp.tile([128, L], F32, tag="scores")
                for l in range(L):
                    if l < n_stt:
                        # fused multiply+reduce on DVE (1x)
                        nc.vector.scalar_tensor_tensor(
                            out=scratch[:],
                            in0=xt[l][:],
                            scalar=1.0,
                            in1=qk_sb[:],
                            op0=ALU.mult,
                            op1=ALU.mult,
                            accum_out=scores[:, l : l + 1],
                        )
                    else:
                        # bf16 2x multiply on DVE, reduce on ACT
                        mr = mulp.tile([128, D], BF, tag="mr")
                        nc.vector.tensor_mul(mr[:], xt[l][:], qk_sb[:])
                        nc.scalar.activation(
                            act_dump[:], mr[:], AF.Identity,
                            accum_out=scores[:, l : l + 1],
                        )

                # softmax over l + double scaling + renorm
                u = smp.tile([128, L], F32, tag="u")
                Ucol = smp.tile([128, 1], F32, tag="Ucol")
                nc.scalar.activation(u[:], scores[:], AF.Exp, accum_out=Ucol[:])
                v = smp.tile([128, L], F32, tag="v")
                nc.vector.tensor_mul(v[:], u[:], sc_sb[:])
                Vcol = smp.tile([128, 1], F32, tag="Vcol")
                nc.vector.tensor_reduce(Vcol[:], v[:], axis=AX.X, op=ALU.add)
                den = smp.tile([128, 1], F32, tag="den")
                nc.vector.tensor_scalar(
                    out=den[:], in0=Ucol[:], scalar1=1e-6, scalar2=Vcol[:],
                    op0=ALU.mult, op1=ALU.add,
                )
                rden = smp.tile([128, 1], F32, tag="rden")
                nc.vector.reciprocal(rden[:], den[:])
                w_t = smp.tile([128, L], F32, tag="w")
                nc.vector.scalar_tensor_tensor(
                    out=w_t[:], in0=v[:], scalar=rden[:], in1=sc_sb[:],
                    op0=ALU.mult, op1=ALU.mult,
                )

                dg_all = diagp.tile([128, L, 128], BF, tag="diag")
                (nc.vector if last else nc.gpsimd).tensor_tensor(
                    out=dg_all[:],
                    in0=id_sb[:].unsqueeze(1).broadcast_to((128, L, 128)),
                    in1=w_t[:].unsqueeze(2).broadcast_to((128, L, 128)),
                    op=ALU.mult,
                )
                op_ = o_ps.tile([128, D], F32)
                for l in range(L):
                    for h in range(2):
                        nc.tensor.matmul(
                            op_[:, h * 512 : (h + 1) * 512],
                            dg_all[:, l, :],
                            xt[l][:, h * 512 : (h + 1) * 512],
                            start=(l == 0),
                            stop=(l == L - 1),
                        )

                # LayerNorm
                s1 = smp.tile([128, 1], F32, tag="s1")
                nc.scalar.activation(scratch2[:], op_[:], AF.Identity, accum_out=s1[:])
                s2 = smp.tile([128, 1], F32, tag="s2")
                nc.scalar.activation(scratch2[:], op_[:], AF.Square, accum_out=s2[:])
                m1 = smp.tile([128, 1], F32, tag="m1")
                nc.vector.tensor_scalar(
                    out=m1[:], in0=s1[:], scalar1=1.0 / D, scalar2=None, op0=ALU.mult
                )
                q2 = smp.tile([128, 1], F32, tag="q2")
                nc.vector.tensor_scalar(
                    out=q2[:], in0=s2[:], scalar1=1.0 / D, scalar2=None, op0=ALU.mult
                )
                m1sq = smp.tile([128, 1], F32, tag="m1sq")
                nc.vector.tensor_scalar(
                    out=m1sq[:], in0=m1[:], scalar1=m1[:], scalar2=None, op0=ALU.mult
                )
                var_ = smp.tile([128, 1], F32, tag="var")
                nc.vector.scalar_tensor_tensor(
                    out=var_[:], in0=q2[:], scalar=1e-5, in1=m1sq[:],
                    op0=ALU.add, op1=ALU.subtract,
                )
                rstd = smp.tile([128, 1], F32, tag="rstd")
                ti = smp.tile([128, 1], I32, tag="nt_ti")
                nc.vector.tensor_scalar(
                    out=ti[:], in0=var_[:].bitcast(I32), scalar1=1,
                    scalar2=None, op0=ALU.arith_shift_right,
                )
                nc.vector.scalar_tensor_tensor(
                    out=rstd[:].bitcast(I32), in0=magic[:], scalar=0.0,
                    in1=ti[:], op0=ALU.bypass, op1=ALU.subtract,
                )
                for _ in range(2):
                    a_ = smp.tile([128, 1], F32, tag="nt_a")
                    nc.vector.tensor_scalar(
                        out=a_[:], in0=rstd[:], scalar1=rstd[:], scalar2=None,
                        op0=ALU.mult,
                    )
                    b_ = smp.tile([128, 1], F32, tag="nt_b")
                    nc.vector.scalar_tensor_tensor(
                        out=b_[:], in0=var_[:], scalar=-0.5, in1=a_[:],
                        op0=ALU.mult, op1=ALU.mult,
                    )
                    c_ = smp.tile([128, 1], F32, tag="nt_c")
                    nc.vector.tensor_scalar(
                        out=c_[:], in0=b_[:], scalar1=1.5, scalar2=None, op0=ALU.add
                    )
                    rstd_new = smp.tile([128, 1], F32, tag="nt_y")
                    nc.vector.tensor_scalar(
                        out=rstd_new[:], in0=rstd[:], scalar1=c_[:], scalar2=None,
                        op0=ALU.mult,
                    )
                    rstd = rstd_new
                nmr = smp.tile([128, 1], F32, tag="nmr")
                nc.vector.tensor_scalar(
                    out=nmr[:], in0=m1[:], scalar1=rstd[:], scalar2=-1.0,
                    op0=ALU.mult, op1=ALU.mult,
                )
                if last:
                    # short drain path: all on DVE
                    t1 = zp.tile([128, D], F32)
                    nc.vector.scalar_tensor_tensor(
                        out=t1[:], in0=op_[:], scalar=m1[:], in1=lnw_sb[:],
                        op0=ALU.subtract, op1=ALU.mult,
                    )
                    t2 = yp.tile([128, D], F32, tag="y1")
                    nc.vector.tensor_scalar(
                        out=t2[:], in0=t1[:], scalar1=rstd[:], scalar2=None,
                        op0=ALU.mult,
                    )
                    nc.vector.tensor_add(t2[:], t2[:], lnb_sb[:])
                    nc.sync.dma_start(y_d[r0 : r0 + 128, :], t2[:])
                else:
                    z = zp.tile([128, D], F32)
                    nc.scalar.activation(z[:], op_[:], AF.Identity, scale=rstd[:], bias=nmr[:])
                    y1 = yp.tile([128, D], F32, tag="y1")
                    nc.gpsimd.tensor_mul(y1[:], z[:], lnw_sb[:])
                    nc.gpsimd.tensor_add(y1[:], y1[:], lnb_sb[:])
                    nc.sync.dma_start(y_d[r0 : r0 + 128, :], y1[:])

            q = [front(0)]
            # late consts (not needed until the first back())
            lnw_sb = constp.tile([128, D], F32)
            nc.sync.dma_start(lnw_sb[:], lnw_d[:])
            lnb_sb = constp.tile([128, D], F32)
            nc.sync.dma_start(lnb_sb[:], lnb_d[:])
            q.append(front(1))
            for b in range(NBLK):
                if b + 2 < NBLK:
                    q.append(front(b + 2))
                back(b, *q.pop(0), last=(b == NBLK - 1))

    nc.compile()
    return nc


MODE = "f16"

_NC_CACHE = {}


def _get_nc(key=None):
    if key is None:
        key = MODE
    if key not in _NC_CACHE:
        if key == "f16":
            _NC_CACHE[key] = build_nc_f16()
        else:
            _NC_CACHE[key] = build_nc(qk_f32r=True, ws_f32r=True)
    return _NC_CACHE[key]


def make_in_maps(layer_outputs, w_q, w_k, ca_scales, ca_temp, ln_w, ln_b, mode=None):
    if mode is None:
        mode = MODE
    M = (w_q.T.astype(np.float32) @ w_k.astype(np.float32)) / np.float32(
        np.sqrt(D) * abs(float(ca_temp[0]))
    )
    M = np.ascontiguousarray(M.astype(np.float32))
    x = np.ascontiguousarray(layer_outputs.reshape(L, B * T, D))
    sc_b = np.ascontiguousarray(np.tile(ca_scales.astype(np.float32), (128, 1)))
    ident = np.eye(128, dtype=np.float32)
    lnw_b = np.ascontiguousarray(np.tile(ln_w.astype(np.float32), (128, 1)))
    lnb_b = np.ascontiguousarray(np.tile(ln_b.astype(np.float32), (128, 1)))
    if mode == "f16":
        x = x.astype(np.float16)
        M = M.astype(np.float16)
    in_maps = []
    for c in range(N_CORES):
        im = {
            "x": np.ascontiguousarray(x[:, c * NT : (c + 1) * NT, :]),
            "m": M,
            "scales_b": sc_b,
            "ident": ident,
            "lnw_b": lnw_b,
            "lnb_b": lnb_b,
        }
        if mode == "f16":
            im["ident_bf"] = ident.astype(np.float16)
        in_maps.append(im)
    return in_maps


def kernel(layer_outputs, w_q, w_k, ca_scales, ca_temp, ln_w, ln_b, **run_kwargs):
    in_maps = make_in_maps(layer_outputs, w_q, w_k, ca_scales, ca_temp, ln_w, ln_b)
    nc = _get_nc()
    res = run_bass_kernel_spmd(nc, in_maps, core_ids=list(range(N_CORES)), **run_kwargs)
    out = np.concatenate([res.results[c]["y"] for c in range(N_CORES)], axis=0)
    return np.ascontiguousarray(out.reshape(B, T, D))
